# revision 17
# baseline (speedup 1.0000x reference)
"""Trainium2 Bass kernel for a dense multi-head attention block.

Full (unsharded) contract: kernel(**inputs) -> np.ndarray [2, 2048, 1024].

Sharding: 8 cores = 2 (batch) x 4 (head-group of 4 heads).  Each core
computes Q/K/V projections for its 4 heads, RoPE, causal attention, and
a partial output (attn_group @ wo_rows).  The 4 partials per batch are
summed on the host (the tensor-parallel unshard).

Device-side layout: activations flow transposed ([dim, seq]) so every
matmul uses naturally-laid-out weights.  Host pre-permutes wq/wk columns
into per-head [even(32) | odd(32)] blocks so RoPE becomes 3 full-width
DVE ops plus one PE block-swap matmul.

Attention runs in two waves of head pairs (low head on partitions 0-63,
high head on 64-127).  Score matmuls for the pair are emitted adjacently
so their K=64 contractions occupy disjoint PE row-groups and overlap.
PV streams the exp arena as the wide moving operand with a per-key-tile
V stationary; a ones column in the stationary yields softmax
denominators as an extra output partition.  The un-normalized PV output
lands directly in [head_dim, seq] layout (no transposes); normalization
is a PE broadcast of the denominator rows + one fused DVE multiply, and
the wo projection + output DMA stream out per 512-query chunk.
"""

import os
import sys
import types

import numpy as np

B, S, D, H = 2, 2048, 1024, 16
HD = D // H          # 64
NHG = 4              # head-groups (tensor-parallel dim)
NH = 4               # heads per core
DHG = 256            # head dims per core
P = 128
N_CORES = 8
KT = D // P          # 8 contraction tiles for the projections
NST = S // P         # 16 sequence tiles

_CACHE = {}


def _install_trace_shim():
    """Make antenv.axon_hooks importable so bass_utils trace=True works."""
    if "antenv.axon_hooks" in sys.modules:
        return
    try:
        import trn_agent_boot.trn_boot as _tb
        hook = _tb._ntff_profile_via_ctypes("/opt/axon/libaxon_pjrt.so")
    except Exception:
        hook = None
    mod = types.ModuleType("antenv.axon_hooks")
    mod.get_axon_ntff_profile_hook = lambda: hook
    mod.set_axon_ntff_profile_hook = lambda h: None
    sys.modules["antenv.axon_hooks"] = mod


def _emit(tc, nc, ap, out_ap, mybir, dbg=None):
    from contextlib import ExitStack

    f32 = mybir.dt.float32
    f16 = mybir.dt.float16
    Exp = mybir.ActivationFunctionType.Exp
    M = mybir.AluOpType.mult

    with ExitStack() as ctx:
        consts = ctx.enter_context(tc.tile_pool(name="consts", bufs=1))

        wo_sb = consts.tile([P, 2, D], f16)
        nc.scalar.dma_start(out=wo_sb, in_=ap["wo"].rearrange("(a p) m -> p a m", p=P))
        pm_sb = consts.tile([P, P], f16)
        nc.scalar.dma_start(out=pm_sb, in_=ap["pm"])
        ident_sb = consts.tile([P, P], f16)
        nc.scalar.dma_start(out=ident_sb, in_=ap["ident"])
        maskd_sb = consts.tile([P, P], f16)
        nc.scalar.dma_start(out=maskd_sb, in_=ap["maskd"])
        sel_sb = consts.tile([P, P], f16)
        nc.scalar.dma_start(out=sel_sb, in_=ap["sel"])

        qrot = consts.tile([P, 2, S], f16)
        krot = consts.tile([P, 2, S], f16)
        # PV stationary per (wave, key tile): low = cols [v_lo(64) | 1],
        # high = cols 65:193 = [0*32 | 1 | 0*31 | v_hi(64)] so the high
        # denominator lands on the 32-aligned output partition 32.
        vst = consts.tile([P, 2, NST, 193], f16)
        nc.vector.memset(vst, 0.0)
        nc.vector.memset(vst[:, :, :, 64:65], 1.0)
        nc.vector.memset(vst[:, :, :, 97:98], 1.0)
        # un-normalized head outputs in [dim, seq] layout, per wave-half
        attnT = consts.tile([P, 2, S], f16)
        # denominator rows: 64 = low head, 32 = high head (rest stay zero)
        den = consts.tile([P, 512], f16)
        nc.vector.memset(den, 0.0)
        # wave-0 wo partials
        obh = consts.tile([P, NST, D], f16)

        # ---------------- stage 1: QKV projections + RoPE ----------------
        with tc.tile_pool(name="s1c", bufs=1) as s1c, \
             tc.tile_pool(name="s1ps", bufs=2, space="PSUM") as s1ps, \
             tc.tile_pool(name="s1tmp", bufs=4) as s1tmp:
            xT_sb = s1c.tile([P, KT, S], f16)
            xTr = ap["xT"].rearrange("(a p) s -> p a s", p=P)
            w_sb = {}
            for wn in ("wq", "wk", "wv"):
                w_sb[wn] = s1c.tile([P, KT, DHG], f16, name=f"w_{wn}", tag=f"w_{wn}")
            for a in range(KT):
                for wn in ("wq", "wk", "wv"):
                    nc.scalar.dma_start(out=w_sb[wn][:, a, :],
                                      in_=ap[wn].rearrange("(a p) m -> p a m", p=P)[:, a, :])
                nc.sync.dma_start(out=xT_sb[:, a, :], in_=xTr[:, a, :])
            cs_sb = {}
            for cn in ("ccq", "ssq", "cck", "ssk"):
                cs_sb[cn] = s1c.tile([P, S], f16, name=f"cs_{cn}", tag=f"cs_{cn}")
                nc.scalar.dma_start(out=cs_sb[cn], in_=ap[cn])

            SC = 512
            pending = []        # (ev, ssl, ccn, ssn, rot, m) awaiting swap+rope

            def emit_swap_rope():
                if not pending:
                    return
                ev, pssl, ccn, ssn, rot, m = pending.pop(0)
                sw = s1ps.tile([P, SC], f32, tag="swap", name="sw")
                nc.tensor.matmul(sw, pm_sb, ev, start=True, stop=True)
                t1 = s1tmp.tile([P, SC], f16, tag="t1", name="t1")
                nc.gpsimd.tensor_mul(t1, ev, cs_sb[ccn][:, pssl])
                t2 = s1tmp.tile([P, SC], f16, tag="t2", name="t2")
                nc.vector.tensor_mul(t2, sw, cs_sb[ssn][:, pssl])
                nc.vector.tensor_add(rot[:, m, pssl], t1, t2)

            qk_specs = (("wq", "ccq", "ssq", qrot), ("wk", "cck", "ssk", krot))
            for sc in range(S // SC):
                ssl = slice(sc * SC, (sc + 1) * SC)
                # a-outer: one pass over xT chunks feeds all 4 q/k accumulators
                prs = {}
                for wn, ccn, ssn, rot in qk_specs:
                    for m in range(2):
                        prs[(wn, m)] = s1ps.tile([P, SC], f32, tag=f"proj{wn}{m}",
                                                 name=f"pr_{wn}_{m}", bufs=1)
                for a in range(KT):
                    for wn, ccn, ssn, rot in qk_specs:
                        for m in range(2):
                            nc.tensor.matmul(
                                prs[(wn, m)],
                                w_sb[wn][:, a, m * P:(m + 1) * P],
                                xT_sb[:, a, ssl],
                                start=(a == 0), stop=(a == KT - 1))
                for wn, ccn, ssn, rot in qk_specs:
                    for m in range(2):
                        ev = s1tmp.tile([P, SC], f16, tag="ev")
                        nc.scalar.copy(ev, prs[(wn, m)])
                        pending.append((ev, ssl, ccn, ssn, rot, m))
                        if len(pending) > 1:
                            emit_swap_rope()
                # V in natural [seq, dim] layout (xT as stationary operand)
                for st in range(SC // P):
                    stg = sc * (SC // P) + st
                    vp = s1ps.tile([P, DHG], f32, tag="vproj")
                    for a in range(KT):
                        nc.tensor.matmul(
                            vp,
                            xT_sb[:, a, stg * P:(stg + 1) * P],
                            w_sb["wv"][:, a, :],
                            start=(a == 0), stop=(a == KT - 1))
                    vpr = vp.rearrange("p (w x) -> p w x", w=2)
                    nc.vector.tensor_copy(vst[:, :, stg, 0:64], vpr[:, :, 0:64])
                    nc.scalar.copy(vst[:, :, stg, 129:193], vpr[:, :, 64:128])
            while pending:
                emit_swap_rope()

        if dbg is not None:
            nc.sync.dma_start(out=dbg["qrot"], in_=qrot)
            nc.sync.dma_start(out=dbg["krot"], in_=krot)
            nc.sync.dma_start(out=dbg["vst"], in_=vst)

        # ------- stage 2: attention in two waves of head pairs -------
        aoff = [0] * (NST + 1)
        for i in range(NST):
            aoff[i + 1] = aoff[i] + (S - i * P)

        for w in range(2):
            with tc.tile_pool(name=f"arp{w}", bufs=1) as arp, \
                 tc.tile_pool(name=f"scps{w}", bufs=2, space="PSUM") as scps, \
                 tc.tile_pool(name=f"pvps{w}", bufs=1, space="PSUM") as pvps, \
                 tc.tile_pool(name=f"tailps{w}", bufs=2, space="PSUM") as tailps, \
                 tc.tile_pool(name=f"smal{w}", bufs=2) as smal:
                arenas = [arp.tile([P, aoff[NST]], f16, name=f"ar{w}_{p_}",
                                   tag=f"ar{p_}") for p_ in range(2)]

                def emit_scores(i):
                    q0 = i * P
                    first = True
                    while q0 < S:
                        n = min(512, S - q0)
                        scrs = []
                        for p_ in range(2):
                            po = 64 * p_
                            scr = scps.tile([P, 512], f32, tag=f"scr{p_}",
                                            name=f"sc{w}_{p_}_{i}_{q0}")
                            nc.tensor.matmul(
                                scr[:, 0:n],
                                krot[po:po + 64, w, i * P:(i + 1) * P],
                                qrot[po:po + 64, w, q0:q0 + n],
                                start=True, stop=not first)
                            scrs.append(scr)
                        if first:
                            # add causal mask onto the diagonal block
                            for p_ in range(2):
                                nc.tensor.matmul(scrs[p_][:, 0:P], maskd_sb,
                                                 ident_sb, start=False, stop=True)
                        a0 = aoff[i] + q0 - i * P
                        for p_ in range(2):
                            nc.scalar.activation(arenas[p_][:, a0:a0 + n],
                                                 scrs[p_][:, 0:n], Exp)
                        first = False
                        q0 += n

                def emit_pv(c):
                    qc = 512 * c
                    last = 4 * c + 3
                    pvt = []
                    for p_ in range(2):
                        pv = pvps.tile([P, 512], f32, tag=f"pv{p_}",
                                       name=f"pv{w}_{p_}_{c}")
                        for i in range(last + 1):
                            qa = max(i * P, qc)
                            off = qa - qc
                            lhs = (vst[:, w, i, 0:65] if p_ == 0
                                   else vst[:, w, i, 65:193])
                            nc.tensor.matmul(
                                pv[0:65, off:512] if p_ == 0 else pv[:, off:512],
                                lhs,
                                arenas[p_][:, aoff[i] + qa - i * P:
                                            aoff[i] + qa - i * P + 512 - off],
                                start=(i == 0), stop=(i == last))
                        pvt.append(pv)
                    nc.vector.tensor_copy(den[64:65, :], pvt[0][64:65, :])
                    nc.vector.tensor_copy(den[32:33, :], pvt[1][32:33, :])
                    return pvt

                def emit_tail_a(c, pvt):
                    qc = 512 * c
                    bc = tailps.tile([P, 512], f32, tag="tail", name=f"bc{w}_{c}")
                    nc.tensor.matmul(bc, sel_sb, den, start=True, stop=True)
                    rcp = smal.tile([P, 512], f32, tag="rcp", name=f"rcp{w}_{c}")
                    nc.vector.reciprocal(rcp, bc)
                    nc.vector.scalar_tensor_tensor(
                        attnT[0:64, w, qc:qc + 512], pvt[0][0:64, :], 1.0,
                        rcp[0:64, :], M, M)
                    nc.vector.scalar_tensor_tensor(
                        attnT[64:128, w, qc:qc + 512], pvt[1][64:128, :], 1.0,
                        rcp[64:128, :], M, M)

                def emit_tail_b(c):
                    for st in range(4 * c, 4 * c + 4):
                        for nn2 in range(2):
                            wop = tailps.tile([P, 512], f32, tag="tail",
                                              name=f"wo{w}_{st}_{nn2}")
                            nc.tensor.matmul(
                                wop, attnT[:, w, st * P:(st + 1) * P],
                                wo_sb[:, w, nn2 * 512:(nn2 + 1) * 512],
                                start=True, stop=True)
                            if w == 0:
                                dst = obh[:, st, nn2 * 512:(nn2 + 1) * 512]
                                if nn2 == 0:
                                    nc.scalar.copy(dst, wop)
                                else:
                                    nc.vector.tensor_copy(dst, wop)
                            else:
                                ob = smal.tile([P, 512], f16, tag="ob",
                                               name=f"ob{st}_{nn2}")
                                nc.vector.tensor_add(
                                    ob, wop,
                                    obh[:, st, nn2 * 512:(nn2 + 1) * 512])
                                nc.sync.dma_start(
                                    out=out_ap[st * P:(st + 1) * P,
                                               nn2 * 512:(nn2 + 1) * 512],
                                    in_=ob)

                pvd = {}
                for g in range(4):
                    for i in range(4 * g, 4 * g + 4):
                        emit_scores(i)
                    if g >= 2:
                        emit_tail_a(g - 2, pvd[g - 2])
                    if g >= 1:
                        pvd[g - 1] = emit_pv(g - 1)
                    if g >= 2:
                        emit_tail_b(g - 2)
                emit_tail_a(2, pvd[2])
                pvd[3] = emit_pv(3)
                emit_tail_b(2)
                emit_tail_a(3, pvd[3])
                emit_tail_b(3)

                if dbg is not None and w == 0:
                    nc.sync.dma_start(out=dbg["arena0"], in_=arenas[0])
                    nc.sync.dma_start(out=dbg["arena1"], in_=arenas[1])

        if dbg is not None:
            nc.sync.dma_start(out=dbg["attnT"], in_=attnT)
            nc.sync.dma_start(out=dbg["obh"], in_=obh)


def _build_program(debug=False):
    import concourse.tile as tile
    import concourse.mybir as mybir
    from concourse import bacc

    f32 = mybir.dt.float32
    f16 = mybir.dt.float16

    nc = bacc.Bacc("TRN2", target_bir_lowering=False, debug=False,
                   num_devices=N_CORES)
    ap = {}

    def inp(name, shape, dt=f32):
        ap[name] = nc.dram_tensor(name, shape, dt, kind="ExternalInput").ap()

    inp("xT", [D, S], f16)
    inp("wq", [D, DHG], f16)
    inp("wk", [D, DHG], f16)
    inp("wv", [D, DHG], f16)
    inp("wo", [DHG, D], f16)
    inp("ccq", [P, S], f16)
    inp("ssq", [P, S], f16)
    inp("cck", [P, S], f16)
    inp("ssk", [P, S], f16)
    inp("maskd", [P, P], f16)
    inp("pm", [P, P], f16)
    inp("ident", [P, P], f16)
    inp("sel", [P, P], f16)
    out_ap = nc.dram_tensor("out", [S, D], f16, kind="ExternalOutput").ap()
    dbg = None
    if debug:
        naoff = S * NST - P * (NST - 1) * NST // 2
        dbg = {
            "qrot": nc.dram_tensor("dbg_qrot", [P, 2, S], f16, kind="ExternalOutput").ap(),
            "krot": nc.dram_tensor("dbg_krot", [P, 2, S], f16, kind="ExternalOutput").ap(),
            "vst": nc.dram_tensor("dbg_vst", [P, 2, NST, 193], f16, kind="ExternalOutput").ap(),
            "attnT": nc.dram_tensor("dbg_attnT", [P, 2, S], f16, kind="ExternalOutput").ap(),
            "obh": nc.dram_tensor("dbg_obh", [P, NST, D], f16, kind="ExternalOutput").ap(),
            "arena0": nc.dram_tensor("dbg_arena0", [P, naoff], f16, kind="ExternalOutput").ap(),
            "arena1": nc.dram_tensor("dbg_arena1", [P, naoff], f16, kind="ExternalOutput").ap(),
        }

    with tile.TileContext(nc) as tc:
        _emit(tc, nc, ap, out_ap, mybir, dbg=dbg)
    nc.compile()
    return nc


def _host_prep(x, wq, wk, wv, wo, freqs_cos, freqs_sin, mask):
    """Build the 8 per-core input maps."""
    perm = []
    for h in range(NH):
        perm += [HD * h + 2 * j for j in range(HD // 2)]
        perm += [HD * h + 2 * j + 1 for j in range(HD // 2)]
    perm = np.asarray(perm)

    cosT = np.ascontiguousarray(freqs_cos.T).astype(np.float32)   # [32, S]
    sinT = np.ascontiguousarray(freqs_sin.T).astype(np.float32)
    CC = np.tile(cosT, (4, 1))                                    # [128, S]
    SS = np.tile(np.vstack([-sinT, sinT]), (2, 1))                # [128, S]
    ccq, ssq = (CC * 0.125).astype(np.float16), (SS * 0.125).astype(np.float16)
    cck, ssk = CC.astype(np.float16), SS.astype(np.float16)

    swap = np.zeros((P, P), dtype=np.float16)
    for g in range(2):
        for j in range(32):
            swap[64 * g + 32 + j, 64 * g + j] = 1.0
            swap[64 * g + j, 64 * g + 32 + j] = 1.0

    m2 = mask[0, 0]
    maskd = np.clip(m2[0:P, 0:P], -30000.0, 30000.0).astype(np.float16)

    ident = np.eye(P, dtype=np.float16)

    sel = np.zeros((P, P), dtype=np.float16)
    sel[64, 0:64] = 1.0
    sel[32, 64:128] = 1.0

    xT = [np.ascontiguousarray(x[b].T).astype(np.float16) for b in range(B)]

    in_maps = []
    for c in range(N_CORES):
        b, hg = c // NHG, c % NHG
        cols = hg * DHG + np.arange(DHG)
        in_maps.append({
            "xT": xT[b],
            "wq": np.ascontiguousarray(wq[:, hg * DHG + perm]).astype(np.float16),
            "wk": np.ascontiguousarray(wk[:, hg * DHG + perm]).astype(np.float16),
            "wv": np.ascontiguousarray(wv[:, cols]).astype(np.float16),
            "wo": np.ascontiguousarray(wo[cols, :]).astype(np.float16),
            "ccq": ccq, "ssq": ssq, "cck": cck, "ssk": ssk,
            "maskd": maskd, "pm": swap, "ident": ident, "sel": sel,
        })
    return in_maps


def kernel(x, wq, wk, wv, wo, freqs_cos, freqs_sin, mask, start_pos=0, **_):
    import concourse.bass_utils as bass_utils

    x = np.asarray(x, dtype=np.float32)
    wq = np.asarray(wq, dtype=np.float32)
    wk = np.asarray(wk, dtype=np.float32)
    wv = np.asarray(wv, dtype=np.float32)
    wo = np.asarray(wo, dtype=np.float32)
    freqs_cos = np.asarray(freqs_cos, dtype=np.float32)
    freqs_sin = np.asarray(freqs_sin, dtype=np.float32)
    mask = np.asarray(mask, dtype=np.float32)

    trace = bool(int(os.environ.get("BASS_KERNEL_TRACE", "0")))
    if trace:
        _install_trace_shim()
        import concourse.bass_utils as bu
        bu.upload_artifacts = lambda tmpdir: "(upload skipped)"

    debug = bool(int(os.environ.get("BASS_KERNEL_DEBUG", "0")))
    key = "nc_dbg" if debug else "nc"
    if key not in _CACHE:
        _CACHE[key] = _build_program(debug=debug)
    nc = _CACHE[key]

    in_maps = _host_prep(x, wq, wk, wv, wo, freqs_cos, freqs_sin, mask)
    kwargs = {}
    if trace:
        kwargs = dict(trace=True, trace_cores=[0],
                      tmpdir=os.environ.get("BASS_KERNEL_TRACE_DIR", None))
    res = None
    last_exc = None
    for attempt in range(5):
        try:
            res = bass_utils.run_bass_kernel_spmd(
                nc, in_maps, core_ids=list(range(N_CORES)), **kwargs)
            break
        except Exception as e:  # transient NRT device errors recover on retry
            last_exc = e
            import time as _time
            _time.sleep(12)
    if res is None:
        raise last_exc
    _CACHE["last_result"] = res

    out = np.zeros((B, S, D), dtype=np.float32)
    for c in range(N_CORES):
        out[c // NHG] += res.results[c]["out"].astype(np.float32)
    return out


# revision 24
# speedup vs baseline: 1.1721x; 1.1721x over previous
"""Trainium2 Bass kernel for a dense multi-head attention block.

Full (unsharded) contract: kernel(**inputs) -> np.ndarray [2, 2048, 1024].

Sharding: 8 cores = 2 (batch) x 4 (head-group of 4 heads).  Each core
computes Q/K/V projections for its 4 heads, RoPE, causal attention, and
a partial output (attn_group @ wo_rows).  The 4 partials per batch are
summed on the host (the tensor-parallel unshard).

Device-side layout: activations flow transposed ([dim, seq]) so every
matmul uses naturally-laid-out weights.  Host pre-permutes wq/wk columns
into per-head [even(32) | odd(32)] blocks so RoPE becomes 3 full-width
DVE ops plus one PE block-swap matmul.

Attention runs in two waves of head pairs (low head on partitions 0-63,
high head on 64-127).  Score matmuls for the pair are emitted adjacently
so their K=64 contractions occupy disjoint PE row-groups and overlap.
PV streams the exp arena as the wide moving operand with a per-key-tile
V stationary; a ones column in the stationary yields softmax
denominators as an extra output partition.  The un-normalized PV output
lands directly in [head_dim, seq] layout (no transposes); normalization
is a PE broadcast of the denominator rows + one fused DVE multiply, and
the wo projection + output DMA stream out per 512-query chunk.
"""

import os
import sys
import types

import numpy as np

B, S, D, H = 2, 2048, 1024, 16
HD = D // H          # 64
NHG = 4              # head-groups (tensor-parallel dim)
NH = 4               # heads per core
DHG = 256            # head dims per core
P = 128
N_CORES = 8
KT = D // P          # 8 contraction tiles for the projections
NST = S // P         # 16 sequence tiles

_CACHE = {}


def _install_trace_shim():
    """Make antenv.axon_hooks importable so bass_utils trace=True works."""
    if "antenv.axon_hooks" in sys.modules:
        return
    try:
        import trn_agent_boot.trn_boot as _tb
        hook = _tb._ntff_profile_via_ctypes("/opt/axon/libaxon_pjrt.so")
    except Exception:
        hook = None
    mod = types.ModuleType("antenv.axon_hooks")
    mod.get_axon_ntff_profile_hook = lambda: hook
    mod.set_axon_ntff_profile_hook = lambda h: None
    sys.modules["antenv.axon_hooks"] = mod


def _emit(tc, nc, ap, out_ap, mybir, dbg=None):
    from contextlib import ExitStack

    f32 = mybir.dt.float32
    f16 = mybir.dt.float16
    Exp = mybir.ActivationFunctionType.Exp
    M = mybir.AluOpType.mult

    with ExitStack() as ctx:
        consts = ctx.enter_context(tc.tile_pool(name="consts", bufs=1))

        wo_sb = consts.tile([P, 2, D], f16)
        nc.scalar.dma_start(out=wo_sb, in_=ap["wo"].rearrange("(a p) m -> p a m", p=P))
        pm_sb = consts.tile([P, P], f16)
        nc.scalar.dma_start(out=pm_sb, in_=ap["pm"])
        ident_sb = consts.tile([P, P], f16)
        nc.scalar.dma_start(out=ident_sb, in_=ap["ident"])
        maskd_sb = consts.tile([P, P], f16)
        nc.scalar.dma_start(out=maskd_sb, in_=ap["maskd"])
        sel_sb = consts.tile([P, P], f16)
        nc.scalar.dma_start(out=sel_sb, in_=ap["sel"])

        qrot = consts.tile([P, 2, S], f16)
        krot = consts.tile([P, 2, S], f16)
        # PV stationary per (wave, key tile): low = cols [v_lo(64) | 1],
        # high = cols 65:193 = [0*32 | 1 | 0*31 | v_hi(64)] so the high
        # denominator lands on the 32-aligned output partition 32.
        vst = consts.tile([P, 2, NST, 193], f16)
        nc.vector.memset(vst, 0.0)
        nc.vector.memset(vst[:, :, :, 64:65], 1.0)
        nc.vector.memset(vst[:, :, :, 97:98], 1.0)
        # un-normalized head outputs in [dim, seq] layout, per wave-half
        attnT = consts.tile([P, 2, S], f16)
        # denominator rows: 64 = low head, 32 = high head (rest stay zero)
        den = consts.tile([P, 512], f16)
        nc.vector.memset(den, 0.0)

        # ---------------- stage 1: QKV projections + RoPE ----------------
        with tc.tile_pool(name="s1c", bufs=1) as s1c, \
             tc.tile_pool(name="s1ps", bufs=2, space="PSUM") as s1ps, \
             tc.tile_pool(name="s1tmp", bufs=4) as s1tmp:
            xT_sb = s1c.tile([P, KT, S], f16)
            xTr = ap["xT"].rearrange("(a p) s -> p a s", p=P)
            w_sb = {}
            for wn in ("wq", "wk", "wv"):
                w_sb[wn] = s1c.tile([P, KT, DHG], f16, name=f"w_{wn}", tag=f"w_{wn}")
            for a in range(KT):
                for wn in ("wq", "wk", "wv"):
                    nc.scalar.dma_start(out=w_sb[wn][:, a, :],
                                      in_=ap[wn].rearrange("(a p) m -> p a m", p=P)[:, a, :])
                nc.sync.dma_start(out=xT_sb[:, a, :], in_=xTr[:, a, :])
            cs_sb = {}
            for cn in ("ccq", "ssq", "cck", "ssk"):
                cs_sb[cn] = s1c.tile([P, S], f16, name=f"cs_{cn}", tag=f"cs_{cn}")
                nc.scalar.dma_start(out=cs_sb[cn], in_=ap[cn])

            SC = 512
            pending = []        # (ev, ssl, ccn, ssn, rot, m) awaiting swap+rope

            def emit_swap_rope():
                if not pending:
                    return
                ev, pssl, ccn, ssn, rot, m = pending.pop(0)
                sw = s1ps.tile([P, SC], f32, tag="swap", name="sw")
                nc.tensor.matmul(sw, pm_sb, ev, start=True, stop=True)
                t1 = s1tmp.tile([P, SC], f16, tag="t1", name="t1")
                nc.gpsimd.tensor_mul(t1, ev, cs_sb[ccn][:, pssl])
                t2 = s1tmp.tile([P, SC], f16, tag="t2", name="t2")
                nc.vector.tensor_mul(t2, sw, cs_sb[ssn][:, pssl])
                nc.gpsimd.tensor_add(rot[:, m, pssl], t1, t2)

            qk_specs = (("wq", "ccq", "ssq", qrot), ("wk", "cck", "ssk", krot))
            for sc in range(S // SC):
                ssl = slice(sc * SC, (sc + 1) * SC)
                # a-outer: one pass over xT chunks feeds all 4 q/k accumulators
                prs = {}
                for wn, ccn, ssn, rot in qk_specs:
                    for m in range(2):
                        prs[(wn, m)] = s1ps.tile([P, SC], f32, tag=f"proj{wn}{m}",
                                                 name=f"pr_{wn}_{m}", bufs=1)
                for a in range(KT):
                    for wn, ccn, ssn, rot in qk_specs:
                        for m in range(2):
                            nc.tensor.matmul(
                                prs[(wn, m)],
                                w_sb[wn][:, a, m * P:(m + 1) * P],
                                xT_sb[:, a, ssl],
                                start=(a == 0), stop=(a == KT - 1))
                for wn, ccn, ssn, rot in qk_specs:
                    for m in range(2):
                        ev = s1tmp.tile([P, SC], f16, tag="ev")
                        nc.scalar.copy(ev, prs[(wn, m)])
                        pending.append((ev, ssl, ccn, ssn, rot, m))
                        if len(pending) > 1:
                            emit_swap_rope()
                # V in natural [seq, dim] layout (xT as stationary operand)
                for st in range(SC // P):
                    stg = sc * (SC // P) + st
                    vp = s1ps.tile([P, DHG], f32, tag="vproj")
                    for a in range(KT):
                        nc.tensor.matmul(
                            vp,
                            xT_sb[:, a, stg * P:(stg + 1) * P],
                            w_sb["wv"][:, a, :],
                            start=(a == 0), stop=(a == KT - 1))
                    vpr = vp.rearrange("p (w x) -> p w x", w=2)
                    nc.vector.tensor_copy(vst[:, :, stg, 0:64], vpr[:, :, 0:64])
                    nc.scalar.copy(vst[:, :, stg, 129:193], vpr[:, :, 64:128])
            while pending:
                emit_swap_rope()

        if dbg is not None:
            nc.sync.dma_start(out=dbg["qrot"], in_=qrot)
            nc.sync.dma_start(out=dbg["krot"], in_=krot)
            nc.sync.dma_start(out=dbg["vst"], in_=vst)

        # ------- stage 2: attention in two waves of head pairs -------
        aoff = [0] * (NST + 1)
        for i in range(NST):
            aoff[i + 1] = aoff[i] + (S - i * P)

        for w in range(2):
            with tc.tile_pool(name=f"arp{w}", bufs=1) as arp, \
                 tc.tile_pool(name=f"scps{w}", bufs=2, space="PSUM") as scps, \
                 tc.tile_pool(name=f"pvps{w}", bufs=1, space="PSUM") as pvps, \
                 tc.tile_pool(name=f"tailps{w}", bufs=2, space="PSUM") as tailps, \
                 tc.tile_pool(name=f"smal{w}", bufs=2) as smal:
                arenas = [arp.tile([P, aoff[NST]], f16, name=f"ar{w}_{p_}",
                                   tag=f"ar{p_}") for p_ in range(2)]

                # Deferred PE work (pv chains, denominator tail, wo mms) is
                # drained a few steps at a time between score chunks so the
                # PE stays busy while the scalar engine works through exp.
                filler = []

                def drain(k):
                    for _ in range(min(k, len(filler))):
                        filler.pop(0)()

                def emit_scores(i):
                    q0 = i * P
                    first = True
                    while q0 < S:
                        n = min(512, S - q0)
                        scrs = []
                        for p_ in range(2):
                            po = 64 * p_
                            scr = scps.tile([P, 512], f32, tag=f"scr{p_}",
                                            name=f"sc{w}_{p_}_{i}_{q0}")
                            nc.tensor.matmul(
                                scr[:, 0:n],
                                krot[po:po + 64, w, i * P:(i + 1) * P],
                                qrot[po:po + 64, w, q0:q0 + n],
                                start=True, stop=not first)
                            scrs.append(scr)
                        if first:
                            # add causal mask onto the diagonal block
                            for p_ in range(2):
                                nc.tensor.matmul(scrs[p_][:, 0:P], maskd_sb,
                                                 ident_sb, start=False, stop=True)
                        a0 = aoff[i] + q0 - i * P
                        for p_ in range(2):
                            nc.scalar.activation(arenas[p_][:, a0:a0 + n],
                                                 scrs[p_][:, 0:n], Exp)
                        first = False
                        q0 += n
                        drain(3)

                def gen_pv_tail(c):
                    qc = 512 * c
                    last = 4 * c + 3
                    pvt = []
                    for p_ in range(2):
                        pv = pvps.tile([P, 512], f32, tag=f"pv{p_}",
                                       name=f"pv{w}_{p_}_{c}")
                        for i2 in range(last + 1):
                            qa = max(i2 * P, qc)
                            off = qa - qc
                            lhs = (vst[:, w, i2, 0:65] if p_ == 0
                                   else vst[:, w, i2, 65:193])
                            o_ap = pv[0:65, off:512] if p_ == 0 else pv[:, off:512]
                            a_ap = arenas[p_][:, aoff[i2] + qa - i2 * P:
                                              aoff[i2] + qa - i2 * P + 512 - off]
                            yield (lambda o=o_ap, l=lhs, a=a_ap,
                                   s=(i2 == 0), t=(i2 == last):
                                   nc.tensor.matmul(o, l, a, start=s, stop=t))
                        pvt.append(pv)

                    def taila():
                        nc.vector.tensor_copy(den[64:65, :], pvt[0][64:65, :])
                        nc.vector.tensor_copy(den[32:33, :], pvt[1][32:33, :])
                        bc = tailps.tile([P, 512], f32, tag="tail",
                                         name=f"bc{w}_{c}")
                        nc.tensor.matmul(bc, sel_sb, den, start=True, stop=True)
                        rcp = smal.tile([P, 512], f32, tag="rcp",
                                        name=f"rcp{w}_{c}")
                        nc.vector.reciprocal_approx_fast(rcp, bc)
                        nc.vector.scalar_tensor_tensor(
                            attnT[0:64, w, qc:qc + 512], pvt[0][0:64, :], 1.0,
                            rcp[0:64, :], M, M)
                        nc.vector.scalar_tensor_tensor(
                            attnT[64:128, w, qc:qc + 512], pvt[1][64:128, :],
                            1.0, rcp[64:128, :], M, M)
                    yield taila

                    for st in range(4 * c, 4 * c + 4):
                        for nn2 in range(2):
                            def wo_step(st=st, nn2=nn2):
                                wop = tailps.tile([P, 512], f32, tag="tail",
                                                  name=f"wo{w}_{st}_{nn2}")
                                nc.tensor.matmul(
                                    wop, attnT[:, w, st * P:(st + 1) * P],
                                    wo_sb[:, w, nn2 * 512:(nn2 + 1) * 512],
                                    start=True, stop=True)
                                ob = smal.tile([P, 512], f16, tag="ob",
                                               name=f"ob{w}_{st}_{nn2}")
                                if nn2 == 0:
                                    nc.scalar.copy(ob, wop)
                                else:
                                    nc.vector.tensor_copy(ob, wop)
                                nc.sync.dma_start(
                                    out=out_ap[w, st * P:(st + 1) * P,
                                               nn2 * 512:(nn2 + 1) * 512],
                                    in_=ob)
                            yield wo_step

                for i in range(NST):
                    if i >= 4 and i % 4 == 0:
                        filler.extend(gen_pv_tail(i // 4 - 1))
                    emit_scores(i)
                filler.extend(gen_pv_tail(3))
                while filler:
                    filler.pop(0)()

                if dbg is not None and w == 0:
                    nc.sync.dma_start(out=dbg["arena0"], in_=arenas[0])
                    nc.sync.dma_start(out=dbg["arena1"], in_=arenas[1])

        if dbg is not None:
            nc.sync.dma_start(out=dbg["attnT"], in_=attnT)


def _build_program(debug=False):
    import concourse.tile as tile
    import concourse.mybir as mybir
    from concourse import bacc

    f32 = mybir.dt.float32
    f16 = mybir.dt.float16

    nc = bacc.Bacc("TRN2", target_bir_lowering=False, debug=False,
                   num_devices=N_CORES)
    ap = {}

    def inp(name, shape, dt=f32):
        ap[name] = nc.dram_tensor(name, shape, dt, kind="ExternalInput").ap()

    inp("xT", [D, S], f16)
    inp("wq", [D, DHG], f16)
    inp("wk", [D, DHG], f16)
    inp("wv", [D, DHG], f16)
    inp("wo", [DHG, D], f16)
    inp("ccq", [P, S], f16)
    inp("ssq", [P, S], f16)
    inp("cck", [P, S], f16)
    inp("ssk", [P, S], f16)
    inp("maskd", [P, P], f16)
    inp("pm", [P, P], f16)
    inp("ident", [P, P], f16)
    inp("sel", [P, P], f16)
    out_ap = nc.dram_tensor("out", [2, S, D], f16, kind="ExternalOutput").ap()
    dbg = None
    if debug:
        naoff = S * NST - P * (NST - 1) * NST // 2
        dbg = {
            "qrot": nc.dram_tensor("dbg_qrot", [P, 2, S], f16, kind="ExternalOutput").ap(),
            "krot": nc.dram_tensor("dbg_krot", [P, 2, S], f16, kind="ExternalOutput").ap(),
            "vst": nc.dram_tensor("dbg_vst", [P, 2, NST, 193], f16, kind="ExternalOutput").ap(),
            "attnT": nc.dram_tensor("dbg_attnT", [P, 2, S], f16, kind="ExternalOutput").ap(),
            "arena0": nc.dram_tensor("dbg_arena0", [P, naoff], f16, kind="ExternalOutput").ap(),
            "arena1": nc.dram_tensor("dbg_arena1", [P, naoff], f16, kind="ExternalOutput").ap(),
        }

    with tile.TileContext(nc) as tc:
        _emit(tc, nc, ap, out_ap, mybir, dbg=dbg)
    nc.compile()
    return nc


def _host_prep(x, wq, wk, wv, wo, freqs_cos, freqs_sin, mask):
    """Build the 8 per-core input maps."""
    perm = []
    for h in range(NH):
        perm += [HD * h + 2 * j for j in range(HD // 2)]
        perm += [HD * h + 2 * j + 1 for j in range(HD // 2)]
    perm = np.asarray(perm)

    cosT = np.ascontiguousarray(freqs_cos.T).astype(np.float32)   # [32, S]
    sinT = np.ascontiguousarray(freqs_sin.T).astype(np.float32)
    CC = np.tile(cosT, (4, 1))                                    # [128, S]
    SS = np.tile(np.vstack([-sinT, sinT]), (2, 1))                # [128, S]
    ccq, ssq = (CC * 0.125).astype(np.float16), (SS * 0.125).astype(np.float16)
    cck, ssk = CC.astype(np.float16), SS.astype(np.float16)

    swap = np.zeros((P, P), dtype=np.float16)
    for g in range(2):
        for j in range(32):
            swap[64 * g + 32 + j, 64 * g + j] = 1.0
            swap[64 * g + j, 64 * g + 32 + j] = 1.0

    m2 = mask[0, 0]
    maskd = np.clip(m2[0:P, 0:P], -30000.0, 30000.0).astype(np.float16)

    ident = np.eye(P, dtype=np.float16)

    sel = np.zeros((P, P), dtype=np.float16)
    sel[64, 0:64] = 1.0
    sel[32, 64:128] = 1.0

    xT = [np.ascontiguousarray(x[b].T).astype(np.float16) for b in range(B)]

    in_maps = []
    for c in range(N_CORES):
        b, hg = c // NHG, c % NHG
        cols = hg * DHG + np.arange(DHG)
        in_maps.append({
            "xT": xT[b],
            "wq": np.ascontiguousarray(wq[:, hg * DHG + perm]).astype(np.float16),
            "wk": np.ascontiguousarray(wk[:, hg * DHG + perm]).astype(np.float16),
            "wv": np.ascontiguousarray(wv[:, cols]).astype(np.float16),
            "wo": np.ascontiguousarray(wo[cols, :]).astype(np.float16),
            "ccq": ccq, "ssq": ssq, "cck": cck, "ssk": ssk,
            "maskd": maskd, "pm": swap, "ident": ident, "sel": sel,
        })
    return in_maps


def kernel(x, wq, wk, wv, wo, freqs_cos, freqs_sin, mask, start_pos=0, **_):
    import concourse.bass_utils as bass_utils

    x = np.asarray(x, dtype=np.float32)
    wq = np.asarray(wq, dtype=np.float32)
    wk = np.asarray(wk, dtype=np.float32)
    wv = np.asarray(wv, dtype=np.float32)
    wo = np.asarray(wo, dtype=np.float32)
    freqs_cos = np.asarray(freqs_cos, dtype=np.float32)
    freqs_sin = np.asarray(freqs_sin, dtype=np.float32)
    mask = np.asarray(mask, dtype=np.float32)

    trace = bool(int(os.environ.get("BASS_KERNEL_TRACE", "0")))
    if trace:
        _install_trace_shim()
        import concourse.bass_utils as bu
        bu.upload_artifacts = lambda tmpdir: "(upload skipped)"

    debug = bool(int(os.environ.get("BASS_KERNEL_DEBUG", "0")))
    key = "nc_dbg" if debug else "nc"
    if key not in _CACHE:
        _CACHE[key] = _build_program(debug=debug)
    nc = _CACHE[key]

    in_maps = _host_prep(x, wq, wk, wv, wo, freqs_cos, freqs_sin, mask)
    kwargs = {}
    if trace:
        kwargs = dict(trace=True, trace_cores=[0],
                      tmpdir=os.environ.get("BASS_KERNEL_TRACE_DIR", None))
    res = None
    last_exc = None
    for attempt in range(5):
        try:
            res = bass_utils.run_bass_kernel_spmd(
                nc, in_maps, core_ids=list(range(N_CORES)), **kwargs)
            break
        except Exception as e:  # transient NRT device errors recover on retry
            last_exc = e
            import time as _time
            _time.sleep(12)
    if res is None:
        raise last_exc
    _CACHE["last_result"] = res

    out = np.zeros((B, S, D), dtype=np.float32)
    for c in range(N_CORES):
        out[c // NHG] += res.results[c]["out"].astype(np.float32).sum(0)
    return out
